# revision 27
# baseline (speedup 1.0000x reference)
"""Trainium2 Bass kernel for nn_Attention (B=2, H=16, S=2048, D=64).

reference:
    scores = einsum("bhqd,bhkd->bhqk", q, k) / sqrt(64)
    scores = where(mask==0, -1e9, scores)          # mask: [B,1,S,S] int32 0/1
    p_attn = softmax(scores, axis=-1)              # [B,H,S,S] fp32  (output)
    out    = einsum("bhqk,bhkd->bhqd", p_attn, v)  # [B,H,S,D] fp32  (output)

Sharding: B*H = 32 head-slices over 8 cores -> 4 heads/core, all of the
same batch, so each core needs only its batch's mask.

Per-core algorithm (no max-subtraction needed: scores ~ N(0,1) after the
1/8 scale, exp never overflows; masked entries underflow to exactly 0):
  - qT/kT [64, 2048] bf16 built by PE transposes of the natural [128,64]
    tiles (fp32 in, bf16 out of the PSUM->SBUF copy).
  - scores psum[128,1024] = qT_tile^T @ kT (bf16 matmul, fp32 accum)
                          + (8192*I)^T @ m01  (mask as a matmul accumulate)
  - e = exp(0.125*psum - 1024) on ACT -> bf16, fused row-sum accum_out.
    unmasked: exp(s/8); masked: exp(s/8 - 1024) == 0.
  - r = 1/sum (DVE); p = e * r (DVE tensor_scalar, bf16, 4x mode)
  - p -> DRAM fp32 via SWDGE cast-DMA;  p -> pT strips via DMA-xbar
    transpose (bf16, SBUF->SBUF)
  - out^T psum[64,512] = sum_t v[t]^T @ pT[t]  (bf16), PE-transposed back
    to [qi, 64] and written out.
"""

import math
from contextlib import ExitStack

import numpy as np

import concourse.bass as bass
import concourse.bacc as bacc
import concourse.tile as tile
import concourse.mybir as mybir
from concourse.bass_utils import run_bass_kernel_spmd
from concourse.masks import make_identity

B, H, S, D = 2, 16, 2048, 64
N_CORES = 8
HEADS_PER_CORE = (B * H) // N_CORES  # 4
P = 128                      # partitions / q-tile rows
N_QT = S // P                # 16 q-tiles (also 16 kj-tiles)
GROUP = 4                    # q-tiles per PV group (qi chunk of 512)
N_GROUPS = N_QT // GROUP
MASK_SCALE = 8192.0          # exact in bf16; 8192/8 = 1024 shift
ACT_BIAS = -1024.0

f32 = mybir.dt.float32
bf16 = mybir.dt.bfloat16
fp8e5 = mybir.dt.float8e5
i32 = mybir.dt.int32


def build_attention_nc(n_heads=HEADS_PER_CORE, n_qt=N_QT, *, cfg=None):
    """Build the single-core Bass program (SPMD: same program, per-core data)."""
    cfg = dict(cfg or {})
    EPOOL_BUFS = cfg.get("epool_bufs", 3)
    PPOOL_BUFS = cfg.get("ppool_bufs", 3)
    STRIP_BUFS = cfg.get("strip_bufs", 3)
    PS_S_BUFS = cfg.get("ps_s_bufs", 3)
    P_OUT_F32 = cfg.get("p_out_f32", False)  # p_i fp32 + HWDGE out (no cast DMA)
    SKIP = cfg.get("skip", set())
    BENCH_ITERS = cfg.get("bench_iters", 0)  # >0: dummy I/O + For_i repeat
    PE_TILES = cfg.get("pe_tiles", 0)  # 0..4: q-tiles per group transposed on PE
    CHUNK512 = cfg.get("chunk512", False)  # scores psum in 4x[128,512] chunks
    nc = bacc.Bacc("TRN2", target_bir_lowering=False, debug=False)

    if BENCH_ITERS:
        io_kind_in = io_kind_out = "Internal"
        dummy_in = nc.dram_tensor("dummy_in", [1, 4], f32, kind="ExternalInput").ap()
        dummy_out = nc.dram_tensor("dummy_out", [1, 4], f32, kind="ExternalOutput").ap()
    else:
        io_kind_in, io_kind_out = "ExternalInput", "ExternalOutput"
    q_d = nc.dram_tensor("q", [n_heads, S, D], f32, kind=io_kind_in).ap()
    k_d = nc.dram_tensor("k", [n_heads, S, D], f32, kind=io_kind_in).ap()
    v_d = nc.dram_tensor("v", [n_heads, S, D], f32, kind=io_kind_in).ap()
    m_d = nc.dram_tensor("mask", [S, S], i32, kind=io_kind_in).ap()
    p_d = nc.dram_tensor("p_attn", [n_heads, S, S], f32, kind=io_kind_out).ap()
    o_d = nc.dram_tensor("out", [n_heads, S, D], f32, kind=io_kind_out).ap()

    with tile.TileContext(nc) as tc, ExitStack() as ctx:
        if BENCH_ITERS:
            dpool = ctx.enter_context(tc.tile_pool(name="dpool", bufs=1))
            dtile = dpool.tile([1, 4], f32)
            nc.sync.dma_start(out=dtile, in_=dummy_in)
            # init internal DRAM to benign values: q/k/v = 0, mask = 1
            zt = dpool.tile([P, N_QT, D], f32)
            nc.vector.memset(zt, 0.0)
            ot = dpool.tile([P, S], i32)
            nc.gpsimd.memset(ot, 1)
            for hh in range(n_heads):
                for td in (q_d, k_d, v_d):
                    nc.sync.dma_start(
                        out=td[hh].rearrange("(t p) d -> p t d", p=P), in_=zt
                    )
            for tt_ in range(N_QT):
                nc.sync.dma_start(out=m_d[tt_ * P:(tt_ + 1) * P, :], in_=ot)
            loop_cm = tc.For_i(0, BENCH_ITERS, 1)
            loop_cm.__enter__()
        singles = ctx.enter_context(tc.tile_pool(name="singles", bufs=1))
        mpool = ctx.enter_context(tc.tile_pool(name="mpool", bufs=1))
        qkpool = ctx.enter_context(tc.tile_pool(name="qkpool", bufs=2))
        natpool = ctx.enter_context(tc.tile_pool(name="natpool", bufs=2))
        epool = ctx.enter_context(tc.tile_pool(name="epool", bufs=EPOOL_BUFS))
        ppool = ctx.enter_context(tc.tile_pool(name="ppool", bufs=PPOOL_BUFS))
        strip_pool = ctx.enter_context(tc.tile_pool(name="strip_pool", bufs=STRIP_BUFS))
        small = ctx.enter_context(tc.tile_pool(name="small", bufs=8))
        opool = ctx.enter_context(tc.tile_pool(name="opool", bufs=2))
        ps_s = ctx.enter_context(tc.tile_pool(name="ps_s", bufs=PS_S_BUFS, space="PSUM"))
        ps_pv = ctx.enter_context(tc.tile_pool(name="ps_pv", bufs=2, space="PSUM"))

        # per-partition bias AP for the exp activation
        bias_t = singles.tile([P, 1], f32)
        nc.vector.memset(bias_t, ACT_BIAS)
        # fp32 identity for PE transposes
        ident = singles.tile([P, P], f32)
        make_identity(nc, ident)
        # plain bf16 identity for PE strip transposes (hybrid mode)
        ident_b = singles.tile([P, P], bf16)
        make_identity(nc, ident_b)
        # 8192 * I: stationary operand of the mask-accumulate matmul
        m_dt = fp8e5 if cfg.get("m01_fp8", True) else bf16
        iscaled = singles.tile([P, P], m_dt)
        nc.gpsimd.memset(iscaled, 0.0)
        nc.gpsimd.affine_select(
            out=iscaled, in_=iscaled,
            compare_op=mybir.AluOpType.not_equal,
            fill=MASK_SCALE, base=0,
            pattern=[[-1, P]], channel_multiplier=1,
        )

        # mask 0/1 int32 -> bf16, resident for the whole core
        m01 = mpool.tile([P, N_QT, S], m_dt)  # [p, qtile, kj]
        for mt in range(4):
            nc.gpsimd.dma_start(
                out=m01[:, mt * 4:(mt + 1) * 4, :],
                in_=m_d[mt * 4 * P:(mt + 1) * 4 * P, :].rearrange(
                    "(t p) k -> p t k", p=P
                ),
            )

        for h in range(n_heads):
            # ---- per-head setup: qT/kT (bf16 [64, S]) and v (bf16) ----
            q_nat = natpool.tile([P, N_QT, D], f32, tag="nat")
            k_nat = natpool.tile([P, N_QT, D], f32, tag="nat")
            nc.sync.dma_start(out=q_nat, in_=q_d[h].rearrange("(t p) d -> p t d", p=P))
            nc.sync.dma_start(out=k_nat, in_=k_d[h].rearrange("(t p) d -> p t d", p=P))
            v_bf = natpool.tile([P, N_QT, D], bf16, tag="vbf")
            nc.gpsimd.dma_start(out=v_bf, in_=v_d[h].rearrange("(t p) d -> p t d", p=P))

            qT = qkpool.tile([D, S], bf16, tag="qT")
            kT = qkpool.tile([D, S], bf16, tag="kT")
            if cfg.get("qkt_pv_pool"):
                tr_pool, tr_tag, tr_w = ps_pv, "pv", 512
            else:
                tr_pool, tr_tag = ps_s, "s"
                tr_w = S // (4 if CHUNK512 else 2)
            for src_nat, dst in ((q_nat, qT), (k_nat, kT)):
                for half in range(S // tr_w):
                    ps_t = tr_pool.tile([D, tr_w], f32, tag=tr_tag, name="ps_t")
                    for tt in range(tr_w // P):
                        t = half * (tr_w // P) + tt
                        nc.tensor.transpose(
                            ps_t[:, tt * P:(tt + 1) * P], src_nat[:, t, :], ident
                        )
                    nc.vector.tensor_copy(
                        dst[:, half * tr_w:(half + 1) * tr_w], ps_t
                    )

            # ---- main loop over q-tile groups ----
            for g in range(N_GROUPS):
                # strip2[kk, il*16+t, qi] = p_group[qi, il, t*128+kk]
                strip2 = strip_pool.tile([P, GROUP * N_QT, P], bf16)
                if not ({"norm", "exp"} & SKIP):
                    p_group = ppool.tile([P, GROUP, S], bf16)
                if "qk" in SKIP:
                    nc.vector.memset(p_group, 0.0)
                for il in range(GROUP):
                    if "qk" in SKIP:
                        break
                    i = g * GROUP + il
                    e_i = epool.tile([P, S], bf16)
                    if CHUNK512:
                        sums = small.tile([P, 4], f32, tag="sums4")
                        ps_chunks = [
                            ps_s.tile([P, 512], f32, tag="s", name="ps_c")
                            for _ in range(4)
                        ]
                        for cc in range(4):
                            kj = slice(cc * 512, (cc + 1) * 512)
                            nc.tensor.matmul(
                                ps_chunks[cc],
                                lhsT=qT[:, i * P:(i + 1) * P],
                                rhs=kT[:, kj],
                                start=True, stop=False,
                            )
                        for cc in range(4):
                            kj = slice(cc * 512, (cc + 1) * 512)
                            nc.tensor.matmul(
                                ps_chunks[cc],
                                lhsT=iscaled,
                                rhs=m01[:, i, kj.start:kj.stop],
                                start=False, stop=True,
                            )
                        for cc in range(4):
                            nc.scalar.activation(
                                out=e_i[:, cc * 512:(cc + 1) * 512],
                                in_=ps_chunks[cc],
                                func=mybir.ActivationFunctionType.Exp,
                                bias=bias_t, scale=0.125,
                                accum_out=sums[:, cc:cc + 1],
                            )
                        ssum = small.tile([P, 1], f32)
                        nc.vector.reduce_sum(ssum, sums, axis=mybir.AxisListType.X)
                        r_i = small.tile([P, 1], f32)
                        nc.vector.reciprocal(r_i, ssum)
                        nc.vector.tensor_scalar_mul(p_group[:, il, :], e_i, r_i)
                        continue
                    sums = small.tile([P, 2], f32)
                    ps_halves = [
                        ps_s.tile([P, S // 2], f32, tag="s", name="ps_half")
                        for _ in range(2)
                    ]
                    # all 4 QK matmuls with one LDW(qT), then all 4 bias
                    # matmuls with one LDW(iscaled): minimal weight swaps
                    for half in range(2):
                        for c in range(2):
                            cols = slice(c * 512, (c + 1) * 512)
                            kj = slice(half * 1024 + c * 512, half * 1024 + (c + 1) * 512)
                            nc.tensor.matmul(
                                ps_halves[half][:, cols],
                                lhsT=qT[:, i * P:(i + 1) * P],
                                rhs=kT[:, kj],
                                start=True, stop=False,
                            )
                    for half in range(2):
                        for c in range(2):
                            cols = slice(c * 512, (c + 1) * 512)
                            kj = slice(half * 1024 + c * 512, half * 1024 + (c + 1) * 512)
                            nc.tensor.matmul(
                                ps_halves[half][:, cols],
                                lhsT=iscaled,
                                rhs=m01[:, i, kj.start:kj.stop],
                                start=False, stop=True,
                            )
                    if "exp" in SKIP:
                        for half in range(2):
                            nc.vector.tensor_copy(
                                e_i[:, half * 1024:(half + 1) * 1024]
                                if False else e_i[:, half * 1024:half * 1024 + 8],
                                ps_halves[half][:, :8],
                            )
                        continue
                    for half in range(2):
                        nc.scalar.activation(
                            out=e_i[:, half * 1024:(half + 1) * 1024],
                            in_=ps_halves[half],
                            func=mybir.ActivationFunctionType.Exp,
                            bias=bias_t, scale=0.125,
                            accum_out=sums[:, half:half + 1],
                        )
                    if "norm" in SKIP:
                        continue
                    ssum = small.tile([P, 1], f32)
                    nc.vector.tensor_add(ssum, sums[:, 0:1], sums[:, 1:2])
                    r_i = small.tile([P, 1], f32)
                    nc.vector.reciprocal(r_i, ssum)
                    nc.vector.tensor_scalar_mul(p_group[:, il, :], e_i, r_i)
                if "pout" not in SKIP:
                    # p_attn out (bf16 -> fp32 cast in the DMA)
                    nsplit = cfg.get("pout_split", 1)
                    step = GROUP // nsplit
                    for ss in range(nsplit):
                        r0 = g * GROUP * P + ss * step * P
                        nc.gpsimd.dma_start(
                            out=p_d[h, r0:r0 + step * P, :].rearrange(
                                "(i p) k -> p i k", p=P
                            ),
                            in_=p_group[:, ss * step:(ss + 1) * step, :],
                        )
                if "pv" in SKIP:
                    continue
                if cfg.get("xbar_contig"):
                    # contiguous dest: flat[kk, qi*64 + il*16 + t]
                    flat = strip2.rearrange("p a b -> p (a b)")
                    nc.sync.dma_start(out=flat, in_=p_group, transpose=True)
                    sview3 = flat.rearrange("p (q il t) -> p q il t", il=GROUP, t=N_QT)
                    ps_o = ps_pv.tile([D, GROUP * P], f32, tag="pv", name="ps_o")
                    for t in range(N_QT):
                        nc.tensor.matmul(
                            ps_o,
                            lhsT=v_bf[:, t, :],
                            rhs=sview3[:, :, :, t].rearrange("p q il -> p il q"),
                            start=(t == 0), stop=(t == N_QT - 1),
                        )
                    oT = opool.tile([D, GROUP * P], f32, tag="oT")
                    nc.vector.tensor_copy(oT, ps_o)
                    o_sb = opool.tile([P, GROUP, D], f32, tag="osb")
                    for il in range(GROUP):
                        ps_oc = ps_pv.tile([P, D], f32, tag="pv", name="ps_oc")
                        nc.tensor.transpose(
                            ps_oc, oT[:, il * P:(il + 1) * P], ident[:D, :D]
                        )
                        nc.vector.tensor_copy(o_sb[:, il, :], ps_oc)
                    nc.sync.dma_start(
                        out=o_d[h, g * GROUP * P:(g + 1) * GROUP * P, :].rearrange(
                            "(i p) d -> p i d", p=P
                        ),
                        in_=o_sb,
                    )
                    if "pout" not in SKIP:
                        pass
                    continue
                if "xbar" in SKIP:
                    nc.vector.memset(strip2, 0.0)
                elif PE_TILES == 0:
                    if cfg.get("xbar_split"):
                        nc.sync.dma_start(
                            out=strip2[:, :2 * N_QT, :],
                            in_=p_group[:, :2, :], transpose=True,
                        )
                        nc.scalar.dma_start(
                            out=strip2[:, 2 * N_QT:, :],
                            in_=p_group[:, 2:, :], transpose=True,
                        )
                    else:
                        xeng = nc.scalar if (cfg.get("xbar_alt") and (g % 2)) else nc.sync
                        xeng.dma_start(out=strip2, in_=p_group, transpose=True)
                else:
                    nxb = GROUP - PE_TILES
                    if nxb:
                        nc.sync.dma_start(
                            out=strip2[:, :nxb * N_QT, :],
                            in_=p_group[:, :nxb, :],
                            transpose=True,
                        )
                    for il in range(nxb, GROUP):
                        for c in range(4):  # 4 transposes per psum bank fill
                            ps_tr = ps_pv.tile(
                                [P, 4, P], bf16, tag="pv", name="ps_tr"
                            )
                            for tt in range(4):
                                t = c * 4 + tt
                                nc.tensor.matmul(
                                    ps_tr[:, tt, :],
                                    lhsT=p_group[:, il, t * P:(t + 1) * P],
                                    rhs=ident_b,
                                    is_transpose=True,
                                )
                            nc.vector.tensor_copy(
                                strip2[:, il * N_QT + c * 4:il * N_QT + (c + 1) * 4, :],
                                ps_tr,
                            )

                # ---- PV for this group: outT[d, qi] = sum_t v[t]^T @ pT[t] ----
                sview = strip2.rearrange("p (i t) q -> p i t q", t=N_QT)
                ps_o = ps_pv.tile([D, GROUP * P], f32, tag="pv", name="ps_o")
                for t in range(N_QT):
                    nc.tensor.matmul(
                        ps_o,
                        lhsT=v_bf[:, t, :],
                        rhs=sview[:, :, t, :],
                        start=(t == 0), stop=(t == N_QT - 1),
                    )
                oT = opool.tile([D, GROUP * P], f32, tag="oT")
                nc.vector.tensor_copy(oT, ps_o)
                o_sb = opool.tile([P, GROUP, D], f32, tag="osb")
                for il in range(GROUP):
                    ps_oc = ps_pv.tile([P, D], f32, tag="pv", name="ps_oc")
                    nc.tensor.transpose(
                        ps_oc, oT[:, il * P:(il + 1) * P], ident[:D, :D]
                    )
                    nc.vector.tensor_copy(o_sb[:, il, :], ps_oc)
                nc.sync.dma_start(
                    out=o_d[h, g * GROUP * P:(g + 1) * GROUP * P, :].rearrange(
                        "(i p) d -> p i d", p=P
                    ),
                    in_=o_sb,
                )

        if BENCH_ITERS:
            loop_cm.__exit__(None, None, None)
            nc.sync.dma_start(out=dummy_out, in_=dtile)

    nc.compile()
    return nc


_NC_CACHE = {}


def _get_nc():
    key = (HEADS_PER_CORE, N_QT)
    if key not in _NC_CACHE:
        _NC_CACHE[key] = build_attention_nc()
    return _NC_CACHE[key]


def kernel(query, key, value, mask, _trace=False, _return_bench=False):
    """Full-input entry point: shard over 8 cores, run, gather."""
    query = np.asarray(query, dtype=np.float32)
    key = np.asarray(key, dtype=np.float32)
    value = np.asarray(value, dtype=np.float32)
    mask = np.asarray(mask, dtype=np.int32)

    qf = query.reshape(B * H, S, D)
    kf = key.reshape(B * H, S, D)
    vf = value.reshape(B * H, S, D)

    in_maps = []
    for c in range(N_CORES):
        sl = slice(c * HEADS_PER_CORE, (c + 1) * HEADS_PER_CORE)
        b = (c * HEADS_PER_CORE) // H
        in_maps.append({
            "q": np.ascontiguousarray(qf[sl]),
            "k": np.ascontiguousarray(kf[sl]),
            "v": np.ascontiguousarray(vf[sl]),
            "mask": np.ascontiguousarray(mask[b, 0]),
        })

    nc = _get_nc()
    res = run_bass_kernel_spmd(
        nc, in_maps, core_ids=list(range(N_CORES)), trace=_trace,
    )

    p_attn = np.concatenate(
        [res.results[c]["p_attn"] for c in range(N_CORES)], axis=0
    ).reshape(B, H, S, S)
    out = np.concatenate(
        [res.results[c]["out"] for c in range(N_CORES)], axis=0
    ).reshape(B, H, S, D)

    if _return_bench:
        return (out, p_attn), res
    return (out, p_attn)
